# revision 6
# baseline (speedup 1.0000x reference)
"""Trainium2 Bass kernel for nn_Decoder_36953898615460.

recon[B, D] = einsum('lbf,lfd->bd', acts[:n], W[:n]) + sum(bias[:n], 0)

Strategy (row-parallel over F, 8 NeuronCores):
  - Shard the contraction dim F across 8 cores: core r owns F columns
    [r*768, (r+1)*768)  ->  local contraction K_loc = n*768.
  - Host prep (pure layout): acts shard transposed to [K_loc, B] so the
    contraction dim lands on SBUF partitions with contiguous DMA; W shard
    reshaped to [K_loc, D]; bias transposed to [D, n].
  - Per core: partial[D, B] (output transposed: d on partitions) computed
    as fp32r (TF32) matmuls accumulating in PSUM per K-chunk, chunk results
    accumulated into an SBUF fp32 accumulator.
  - B is processed in two halves, each with a full K pass and its own
    ReduceScatter(add); the first RS overlaps the second half's compute,
    so only the second RS is exposed at the tail. (W is streamed twice;
    DMA stays under the PE roofline.)
  - bias: each core adds sum_l(bias)/8 so the 8-way reduce sums to +bias.
  - Device-side ReduceScatter(add): core r ends with rows [r*96, (r+1)*96)
    of the reduced [D, B].
  - Host: concat the 8 shards -> [D, B], transpose -> [B, D].
"""

import numpy as np

import concourse.mybir as mybir
import concourse.tile as tile
from concourse import bacc
from concourse.bass import ts
from concourse.bass_utils import run_bass_kernel_spmd

NCORES = 8
B, F, D = 2048, 6144, 768
F_LOC = F // NCORES  # 768
P = 128
NFREE = 512          # matmul moving free dim (one PSUM bank of fp32)
CK = 6               # k-tiles (of 128) per chunk
HALVES = 2           # B split; each half gets a full K pass + its own RS
BH = B // HALVES     # 1024
PARTIAL_DT = mybir.dt.float32  # wire format for the ReduceScatter

_nc_cache = {}
last_result = None  # BassKernelResults of the most recent run (for test harness)


def _build(n_layers: int):
    K_LOC = n_layers * F_LOC          # 9216 for n=12
    KT = K_LOC // P                   # 72 k-tiles
    assert KT % CK == 0
    NCH = KT // CK                    # 12 chunks
    MD = D // P                       # 6 d-subtiles
    NB = BH // NFREE                  # 2 b-chunks per half
    DR = D // NCORES                  # 96 rows per rank after ReduceScatter

    nc = bacc.Bacc(None, num_devices=NCORES)
    a_ext = nc.dram_tensor("a_t", [K_LOC, B], mybir.dt.float32r, kind="ExternalInput")
    w_ext = nc.dram_tensor("w", [K_LOC, D], mybir.dt.float32r, kind="ExternalInput")
    b_ext = nc.dram_tensor("bias_t", [D, n_layers], mybir.dt.float32, kind="ExternalInput")
    y_ext = nc.dram_tensor("y", [DR, B], PARTIAL_DT, kind="ExternalOutput")

    partials = [
        nc.dram_tensor(f"partial{h}", [D, BH], PARTIAL_DT) for h in range(HALVES)
    ]
    reduceds = [
        nc.dram_tensor(f"reduced{h}", [DR, BH], PARTIAL_DT) for h in range(HALVES)
    ]

    a_v = a_ext[:, :].rearrange("(ko p) b -> p ko b", p=P)  # [128, KT, B]
    w_v = w_ext[:, :].rearrange("(ko p) d -> p ko d", p=P)  # [128, KT, D]
    b_v = b_ext[:, :].rearrange("(mo p) l -> p mo l", p=P)  # [128, MD, n]

    with tile.TileContext(nc) as tc:
        with (
            tc.tile_pool(name="apool", bufs=2) as apool,
            tc.tile_pool(name="wpool", bufs=2) as wpool,
            tc.tile_pool(name="cpool", bufs=1) as cpool,
            tc.tile_pool(name="opool", bufs=2) as opool,
            tc.tile_pool(name="pspool", bufs=3, space="PSUM") as pspool,
        ):
            # bias8[p, mo] = sum_l bias[l, mo*128+p] / NCORES
            bias_t = cpool.tile([P, MD, n_layers], mybir.dt.float32)
            nc.sync.dma_start(bias_t[:], b_v)
            bias8 = cpool.tile([P, MD], mybir.dt.float32)
            nc.vector.reduce_sum(bias8[:], bias_t[:], axis=mybir.AxisListType.X)
            nc.vector.tensor_scalar_mul(bias8[:], bias8[:], 1.0 / NCORES)

            for h in range(HALVES):
                b0 = h * BH
                # fp32 accumulator for this half's partial, acc[p, mo, b]
                acc = cpool.tile([P, MD, BH], mybir.dt.float32, tag="acc")
                for c in range(NCH):
                    a_c = apool.tile([P, CK, BH], mybir.dt.float32r, tag="a")
                    w_c = wpool.tile([P, CK, D], mybir.dt.float32r, tag="w")
                    for k in range(CK):
                        nc.sync.dma_start(
                            a_c[:, k], a_v[:, c * CK + k, b0 : b0 + BH]
                        )
                        nc.sync.dma_start(w_c[:, k], w_v[:, c * CK + k])
                    for m in range(MD):
                        ps = pspool.tile([P, BH], mybir.dt.float32, tag="ps")
                        for k in range(CK):
                            lhsT = w_c[:, k, ts(m, P)]
                            for nb in range(NB):
                                nc.tensor.matmul(
                                    ps[:, ts(nb, NFREE)],
                                    lhsT,
                                    a_c[:, k, ts(nb, NFREE)],
                                    start=(k == 0),
                                    stop=(k == CK - 1),
                                )
                        if c == 0:
                            nc.vector.tensor_scalar_add(
                                acc[:, m], ps[:], bias8[:, m : m + 1]
                            )
                        else:
                            nc.vector.tensor_add(acc[:, m], ps[:], acc[:, m])

                # write this half's partial (convert only if wire dtype differs)
                for m in range(MD):
                    if PARTIAL_DT == mybir.dt.float32:
                        nc.sync.dma_start(partials[h][ts(m, P), :], acc[:, m])
                    else:
                        pb = opool.tile([P, BH], PARTIAL_DT, tag="pb")
                        nc.vector.tensor_copy(pb[:], acc[:, m])
                        nc.sync.dma_start(partials[h][ts(m, P), :], pb[:])

                nc.gpsimd.collective_compute(
                    "ReduceScatter",
                    mybir.AluOpType.add,
                    replica_groups=[list(range(NCORES))],
                    ins=[partials[h][:, :].opt()],
                    outs=[reduceds[h][:, :].opt()],
                )

            # Final output DMAs last, on the SWDGE (gpsimd) queue: a y-DMA
            # waits on its RS completion, and a waiting DMA at the head of
            # the sync HWDGE queue would stall the second half's input
            # streaming behind it (measured 41 us PE gap).
            for h in range(HALVES):
                nc.gpsimd.dma_start(
                    y_ext[:, h * BH : (h + 1) * BH], reduceds[h][:, :]
                )
    nc.compile()
    return nc


def _get_nc(n_layers: int):
    if n_layers not in _nc_cache:
        _nc_cache[n_layers] = _build(n_layers)
    return _nc_cache[n_layers]


def kernel(acts: np.ndarray, W: np.ndarray, bias: np.ndarray, layer_idx) -> np.ndarray:
    global last_result
    n = int(layer_idx) + 1
    acts = np.asarray(acts, dtype=np.float32)[:n]  # [n, B, F]
    W = np.asarray(W, dtype=np.float32)[:n]        # [n, F, D]
    bias = np.asarray(bias, dtype=np.float32)[:n]  # [n, D]

    nc = _get_nc(n)

    bias_t = np.ascontiguousarray(bias.T)  # [D, n], same on every core
    in_maps = []
    for r in range(NCORES):
        f0 = r * F_LOC
        # [n, B, F_LOC] -> [n, F_LOC, B] -> [K_loc, B]
        a_t = np.ascontiguousarray(acts[:, :, f0 : f0 + F_LOC].transpose(0, 2, 1)).reshape(
            n * F_LOC, B
        )
        w_r = np.ascontiguousarray(W[:, f0 : f0 + F_LOC, :]).reshape(n * F_LOC, D)
        in_maps.append({"a_t": a_t, "w": w_r, "bias_t": bias_t})

    last_result = run_bass_kernel_spmd(nc, in_maps, core_ids=list(range(NCORES)))
    out_t = np.concatenate([last_result.results[r]["y"] for r in range(NCORES)], axis=0)
    return np.ascontiguousarray(out_t.T.astype(np.float32))  # [B, D] float32


# revision 7
# speedup vs baseline: 1.0442x; 1.0442x over previous
"""Trainium2 Bass kernel for nn_Decoder_36953898615460.

recon[B, D] = einsum('lbf,lfd->bd', acts[:n], W[:n]) + sum(bias[:n], 0)

Strategy (row-parallel over F, 8 NeuronCores):
  - Shard the contraction dim F across 8 cores: core r owns F columns
    [r*768, (r+1)*768)  ->  local contraction K_loc = n*768.
  - Host prep (pure layout): acts shard transposed to [K_loc, B] so the
    contraction dim lands on SBUF partitions with contiguous DMA; W shard
    reshaped to [K_loc, D]; bias transposed to [D, n].
  - Per core: partial[D, B] (output transposed: d on partitions) computed
    as fp32r (TF32) matmuls accumulating in PSUM per K-chunk, chunk results
    accumulated into an SBUF fp32 accumulator.
  - B is processed in two halves, each with a full K pass and its own
    ReduceScatter(add); the first RS overlaps the second half's compute,
    so only the second RS is exposed at the tail. (W is streamed twice;
    DMA stays under the PE roofline.)
  - bias: each core adds sum_l(bias)/8 so the 8-way reduce sums to +bias.
  - Device-side ReduceScatter(add): core r ends with rows [r*96, (r+1)*96)
    of the reduced [D, B] (bf16 wire).
  - Host: concat the 8 shards -> [D, B], cast bf16->fp32 (exact),
    transpose -> [B, D].
"""

import numpy as np

import concourse.mybir as mybir
import concourse.tile as tile
from concourse import bacc
from concourse.bass import ts
from concourse.bass_utils import run_bass_kernel_spmd

NCORES = 8
B, F, D = 2048, 6144, 768
F_LOC = F // NCORES  # 768
P = 128
NFREE = 512          # matmul moving free dim (one PSUM bank of fp32)
CK = 6               # k-tiles (of 128) per chunk
HALVES = 2           # B split; each half gets a full K pass + its own RS
BH = B // HALVES     # 1024
PARTIAL_DT = mybir.dt.bfloat16  # wire format for the ReduceScatter (bf16: halves RS bytes; rel err ~3e-3 vs fp32 wire ~1.5e-4)

_nc_cache = {}
last_result = None  # BassKernelResults of the most recent run (for test harness)


def _build(n_layers: int):
    K_LOC = n_layers * F_LOC          # 9216 for n=12
    KT = K_LOC // P                   # 72 k-tiles
    assert KT % CK == 0
    NCH = KT // CK                    # 12 chunks
    MD = D // P                       # 6 d-subtiles
    NB = BH // NFREE                  # 2 b-chunks per half
    DR = D // NCORES                  # 96 rows per rank after ReduceScatter

    nc = bacc.Bacc(None, num_devices=NCORES)
    a_ext = nc.dram_tensor("a_t", [K_LOC, B], mybir.dt.float32r, kind="ExternalInput")
    w_ext = nc.dram_tensor("w", [K_LOC, D], mybir.dt.float32r, kind="ExternalInput")
    b_ext = nc.dram_tensor("bias_t", [D, n_layers], mybir.dt.float32, kind="ExternalInput")
    y_ext = nc.dram_tensor("y", [DR, B], PARTIAL_DT, kind="ExternalOutput")

    partials = [
        nc.dram_tensor(f"partial{h}", [D, BH], PARTIAL_DT) for h in range(HALVES)
    ]
    reduceds = [
        nc.dram_tensor(f"reduced{h}", [DR, BH], PARTIAL_DT) for h in range(HALVES)
    ]

    a_v = a_ext[:, :].rearrange("(ko p) b -> p ko b", p=P)  # [128, KT, B]
    w_v = w_ext[:, :].rearrange("(ko p) d -> p ko d", p=P)  # [128, KT, D]
    b_v = b_ext[:, :].rearrange("(mo p) l -> p mo l", p=P)  # [128, MD, n]

    with tile.TileContext(nc) as tc:
        with (
            tc.tile_pool(name="apool", bufs=2) as apool,
            tc.tile_pool(name="wpool", bufs=2) as wpool,
            tc.tile_pool(name="cpool", bufs=1) as cpool,
            tc.tile_pool(name="opool", bufs=2) as opool,
            tc.tile_pool(name="pspool", bufs=3, space="PSUM") as pspool,
        ):
            # bias8[p, mo] = sum_l bias[l, mo*128+p] / NCORES
            bias_t = cpool.tile([P, MD, n_layers], mybir.dt.float32)
            nc.sync.dma_start(bias_t[:], b_v)
            bias8 = cpool.tile([P, MD], mybir.dt.float32)
            nc.vector.reduce_sum(bias8[:], bias_t[:], axis=mybir.AxisListType.X)
            nc.vector.tensor_scalar_mul(bias8[:], bias8[:], 1.0 / NCORES)

            for h in range(HALVES):
                b0 = h * BH
                # fp32 accumulator for this half's partial, acc[p, mo, b]
                acc = cpool.tile([P, MD, BH], mybir.dt.float32, tag="acc")
                for c in range(NCH):
                    a_c = apool.tile([P, CK, BH], mybir.dt.float32r, tag="a")
                    w_c = wpool.tile([P, CK, D], mybir.dt.float32r, tag="w")
                    for k in range(CK):
                        nc.sync.dma_start(
                            a_c[:, k], a_v[:, c * CK + k, b0 : b0 + BH]
                        )
                        nc.sync.dma_start(w_c[:, k], w_v[:, c * CK + k])
                    for m in range(MD):
                        ps = pspool.tile([P, BH], mybir.dt.float32, tag="ps")
                        for k in range(CK):
                            lhsT = w_c[:, k, ts(m, P)]
                            for nb in range(NB):
                                nc.tensor.matmul(
                                    ps[:, ts(nb, NFREE)],
                                    lhsT,
                                    a_c[:, k, ts(nb, NFREE)],
                                    start=(k == 0),
                                    stop=(k == CK - 1),
                                )
                        if c == 0:
                            nc.vector.tensor_scalar_add(
                                acc[:, m], ps[:], bias8[:, m : m + 1]
                            )
                        else:
                            nc.vector.tensor_add(acc[:, m], ps[:], acc[:, m])

                # write this half's partial (convert only if wire dtype differs)
                for m in range(MD):
                    if PARTIAL_DT == mybir.dt.float32:
                        nc.sync.dma_start(partials[h][ts(m, P), :], acc[:, m])
                    else:
                        pb = opool.tile([P, BH], PARTIAL_DT, tag="pb")
                        nc.vector.tensor_copy(pb[:], acc[:, m])
                        nc.sync.dma_start(partials[h][ts(m, P), :], pb[:])

                nc.gpsimd.collective_compute(
                    "ReduceScatter",
                    mybir.AluOpType.add,
                    replica_groups=[list(range(NCORES))],
                    ins=[partials[h][:, :].opt()],
                    outs=[reduceds[h][:, :].opt()],
                )

            # Final output DMAs last, on the SWDGE (gpsimd) queue: a y-DMA
            # waits on its RS completion, and a waiting DMA at the head of
            # the sync HWDGE queue would stall the second half's input
            # streaming behind it (measured 41 us PE gap).
            for h in range(HALVES):
                nc.gpsimd.dma_start(
                    y_ext[:, h * BH : (h + 1) * BH], reduceds[h][:, :]
                )
    nc.compile()
    return nc


def _get_nc(n_layers: int):
    if n_layers not in _nc_cache:
        _nc_cache[n_layers] = _build(n_layers)
    return _nc_cache[n_layers]


def kernel(acts: np.ndarray, W: np.ndarray, bias: np.ndarray, layer_idx) -> np.ndarray:
    global last_result
    n = int(layer_idx) + 1
    acts = np.asarray(acts, dtype=np.float32)[:n]  # [n, B, F]
    W = np.asarray(W, dtype=np.float32)[:n]        # [n, F, D]
    bias = np.asarray(bias, dtype=np.float32)[:n]  # [n, D]

    nc = _get_nc(n)

    bias_t = np.ascontiguousarray(bias.T)  # [D, n], same on every core
    in_maps = []
    for r in range(NCORES):
        f0 = r * F_LOC
        # [n, B, F_LOC] -> [n, F_LOC, B] -> [K_loc, B]
        a_t = np.ascontiguousarray(acts[:, :, f0 : f0 + F_LOC].transpose(0, 2, 1)).reshape(
            n * F_LOC, B
        )
        w_r = np.ascontiguousarray(W[:, f0 : f0 + F_LOC, :]).reshape(n * F_LOC, D)
        in_maps.append({"a_t": a_t, "w": w_r, "bias_t": bias_t})

    last_result = run_bass_kernel_spmd(nc, in_maps, core_ids=list(range(NCORES)))
    out_t = np.concatenate([last_result.results[r]["y"] for r in range(NCORES)], axis=0)
    return np.ascontiguousarray(out_t.T.astype(np.float32))  # [B, D] float32
